# revision 1
# baseline (speedup 1.0000x reference)
"""Trainium2 Bass kernel for nn_Attention_RoPE (LN -> QKV -> RoPE -> attention -> out-proj).

Sharding: 8 cores = 4 batches x 2 head-groups (8 heads each).
Each core computes a partial out-projection [S, D] for its (batch, head-group);
host sums the two partials per batch and adds b_out.

Per-core pipeline (single Bass program, SPMD over 8 cores):
  phase 1+2 (fused, per 128-row seq tile):
    LN (bn_stats/bn_aggr, DVE) -> PE transpose to xnT -> QKV matmuls (bf16)
    -> RoPE on q,k (DVE, from PSUM) -> PE transpose to qT/kT [feat, seq]
    -> v copied to SBUF with an appended ones column (for softmax denominators)
  phase 3 (per head-pair, per 512-query block):
    S^T = K @ Q^T via row-packed matmuls (head0 rows 0:64, head1 rows 64:128)
    exp on ScalarE over [128, 1024] tiles (pair-packed; the throughput bottleneck)
    PV via lhsT=exp(S^T), rhs=[v | ones]  -> denominators for free
    scale by 1/sum, PE transpose into attn_outT [feat, seq]
  phase 4: out-projection, DMA partial result
"""

import numpy as np
import sys

sys.path.insert(0, "/opt/trn_rl_repo")

import ml_dtypes

import concourse.bass as bass
from concourse import bacc
import concourse.mybir as mybir
import concourse.tile as tile
from concourse.masks import make_identity
from concourse.bass_utils import run_bass_kernel_spmd

# Problem constants (hardcoded per contract)
B, S, D = 4, 2048, 1024
H, DH = 16, 64
HG = 2              # head groups (tensor-parallel dim)
NH = H // HG        # heads per core = 8
IN = NH * DH        # per-core inner dim = 512
P = 128
NT = S // P         # 16 seq tiles
NCK = D // P        # 8 contraction chunks
NPAIR = NH // 2     # 4 head pairs
QB = 512            # query block in phase 3
EPS = 1e-5
BASE = 10000.0

F32 = mybir.dt.float32
BF16 = mybir.dt.bfloat16

_CACHE = {}


def _build_nc():
    nc = bacc.Bacc(None, target_bir_lowering=False, debug=False)

    x_d = nc.declare_dram_parameter("x", [S, D], F32, isOutput=False)
    wq_d = nc.declare_dram_parameter("wq", [D, IN], BF16, isOutput=False)
    wk_d = nc.declare_dram_parameter("wk", [D, IN], BF16, isOutput=False)
    wv_d = nc.declare_dram_parameter("wv", [D, IN], BF16, isOutput=False)
    wo_d = nc.declare_dram_parameter("wo", [IN, D], F32, isOutput=False)
    cos_d = nc.declare_dram_parameter("cos_rep", [S, NH * 32], F32, isOutput=False)
    sin_d = nc.declare_dram_parameter("sin_rep", [S, NH * 32], F32, isOutput=False)
    out_d = nc.declare_dram_parameter("out", [S, D], F32, isOutput=True)

    with tile.TileContext(nc) as tc:
        with tc.tile_pool(name="persist", bufs=1) as pers:
            ident = pers.tile([P, P], BF16)
            make_identity(nc, ident)
            eps_t = pers.tile([P, 1], F32)
            nc.vector.memset(eps_t, EPS)

            # weights resident in SBUF
            wq_s = pers.tile([P, NCK, IN], BF16, tag="wq")
            wk_s = pers.tile([P, NCK, IN], BF16, tag="wk")
            wv_s = pers.tile([P, NCK, IN], BF16, tag="wv")
            for w_s, w_d in ((wq_s, wq_d), (wk_s, wk_d), (wv_s, wv_d)):
                nc.sync.dma_start(
                    out=w_s, in_=w_d.rearrange("(c p) n -> p c n", p=P)
                )
            wo_s = pers.tile([P, 4, D], F32, tag="wo")
            nc.sync.dma_start(out=wo_s, in_=wo_d.rearrange("(c p) n -> p c n", p=P))

            # persistent activations
            qT = [pers.tile([P, S], BF16, tag=f"qT{i}", name=f"qT{i}") for i in range(NPAIR)]
            kT = [pers.tile([P, S], BF16, tag=f"kT{i}", name=f"kT{i}") for i in range(NPAIR)]
            # v with ones column appended per head: [kpos, head, 65]
            v_aug = [pers.tile([P, NH, DH + 1], F32, tag=f"v{i}", name=f"v{i}") for i in range(NT)]
            attnT = [pers.tile([P, S], F32, tag=f"aT{i}", name=f"aT{i}") for i in range(NPAIR)]

            # ---------------- phase 1 + 2 ----------------
            with tc.tile_pool(name="ph12", bufs=4) as tp, \
                 tc.tile_pool(name="ph12s", bufs=6) as sp, \
                 tc.tile_pool(name="ps_tr", bufs=2, space="PSUM") as ptr, \
                 tc.tile_pool(name="ps_proj", bufs=2, space="PSUM") as ppr:
                for s in range(NT):
                    r0 = s * P
                    x_s = tp.tile([P, D], F32, tag="x")
                    nc.gpsimd.dma_start(out=x_s, in_=x_d[r0 : r0 + P, :])

                    stats = sp.tile([P, 2, 6], F32, tag="stats")
                    for i in range(2):
                        nc.vector.bn_stats(
                            out=stats[:, i, :], in_=x_s[:, i * 512 : (i + 1) * 512]
                        )
                    mv = sp.tile([P, 2], F32, tag="mv")
                    nc.vector.bn_aggr(out=mv, in_=stats)
                    std = sp.tile([P, 1], F32, tag="std")
                    nc.scalar.activation(
                        out=std, in_=mv[:, 1:2],
                        func=mybir.ActivationFunctionType.Sqrt, bias=eps_t,
                    )
                    rstd = sp.tile([P, 1], F32, tag="rstd")
                    nc.vector.reciprocal(out=rstd, in_=std)

                    xn_s = tp.tile([P, D], BF16, tag="xn")
                    nc.vector.tensor_scalar(
                        out=xn_s, in0=x_s, scalar1=mv[:, 0:1], scalar2=rstd,
                        op0=mybir.AluOpType.subtract, op1=mybir.AluOpType.mult,
                    )

                    # transpose xn -> xnT blocks [dim, seq]
                    xnT_s = tp.tile([P, NCK, P], BF16, tag="xnT")
                    for c in range(NCK):
                        pt = ptr.tile([P, P], BF16, tag="tr")
                        nc.tensor.transpose(pt, xn_s[:, c * P : (c + 1) * P], ident)
                        nc.vector.tensor_copy(out=xnT_s[:, c, :], in_=pt)

                    # projections: q, k, v  [128 seq, 512 feat]
                    ps_q = ppr.tile([P, IN], F32, tag="pq")
                    ps_k = ppr.tile([P, IN], F32, tag="pk")
                    ps_v = ppr.tile([P, IN], F32, tag="pv")
                    for ps, w_s in ((ps_q, wq_s), (ps_k, wk_s), (ps_v, wv_s)):
                        for c in range(NCK):
                            nc.tensor.matmul(
                                ps, lhsT=xnT_s[:, c, :], rhs=w_s[:, c, :],
                                start=(c == 0), stop=(c == NCK - 1),
                            )

                    # v -> SBUF with ones column
                    v_s = v_aug[s]
                    nc.vector.tensor_copy(
                        out=v_s[:, :, 0:DH],
                        in_=ps_v.rearrange("p (h d) -> p h d", h=NH),
                    )
                    nc.vector.memset(v_s[:, :, DH : DH + 1], 1.0)

                    # RoPE on q, k (into bf16 rot tiles)
                    cos_s = sp.tile([P, NH, 32], F32, tag="cos")
                    sin_s = sp.tile([P, NH, 32], F32, tag="sin")
                    nc.gpsimd.dma_start(
                        out=cos_s, in_=cos_d[r0 : r0 + P, :].rearrange("p (h d) -> p h d", h=NH)
                    )
                    nc.gpsimd.dma_start(
                        out=sin_s, in_=sin_d[r0 : r0 + P, :].rearrange("p (h d) -> p h d", h=NH)
                    )
                    for name, ps in (("q", ps_q), ("k", ps_k)):
                        p3 = ps.rearrange("p (h d) -> p h d", h=NH)
                        x1, x2 = p3[:, :, 0:32], p3[:, :, 32:64]
                        rot = tp.tile([P, NH, DH], BF16, tag=f"rot{name}")
                        t1 = sp.tile([P, NH, 32], F32, tag="t1")
                        t2 = sp.tile([P, NH, 32], F32, tag="t2")
                        nc.vector.tensor_mul(t1, x1, cos_s)
                        nc.vector.tensor_mul(t2, x2, sin_s)
                        nc.vector.tensor_sub(rot[:, :, 0:32], t1, t2)
                        nc.vector.tensor_mul(t1, x1, sin_s)
                        nc.vector.tensor_mul(t2, x2, cos_s)
                        nc.vector.tensor_add(rot[:, :, 32:64], t1, t2)
                        # transpose rot -> qT/kT [feat, seq] (2 heads per 128-block)
                        dstl = qT if name == "q" else kT
                        rflat = rot.rearrange("p h d -> p (h d)")
                        for fg in range(NPAIR):
                            pt = ptr.tile([P, P], BF16, tag="tr")
                            nc.tensor.transpose(
                                pt, rflat[:, fg * P : (fg + 1) * P], ident
                            )
                            nc.vector.tensor_copy(
                                out=dstl[fg][:, r0 : r0 + P], in_=pt
                            )

            # ---------------- phase 3: attention ----------------
            scale = 1.0 / np.sqrt(DH)
            with tc.tile_pool(name="pt_pool", bufs=6) as ptp, \
                 tc.tile_pool(name="sc_pool", bufs=3) as scp, \
                 tc.tile_pool(name="sm_pool", bufs=8) as smp, \
                 tc.tile_pool(name="ps_st", bufs=2, space="PSUM") as pst, \
                 tc.tile_pool(name="ps_pv", bufs=1, space="PSUM") as ppv, \
                 tc.tile_pool(name="dram_sc", bufs=8, space="DRAM") as dpool:
                for pair in range(NPAIR):
                    for qb2 in range(2):
                        # two query blocks in flight: independent chains hide sem latency
                        pvTs = {}
                        for j in range(2):
                            for hh in range(2):
                                pvTs[(j, hh)] = ppv.tile(
                                    [DH + 1, QB], F32, tag=f"pvT{j}{hh}", name=f"pvT{j}{hh}"
                                )
                        for kb in range(NT):
                            for j in range(2):
                                q0 = (qb2 * 2 + j) * QB
                                ps_st = pst.tile([P, 2 * QB], F32, tag="st")
                                for hh in range(2):
                                    nc.tensor.matmul(
                                        ps_st[:, hh * QB : (hh + 1) * QB],
                                        lhsT=kT[pair][hh * 64 : (hh + 1) * 64, kb * P : (kb + 1) * P],
                                        rhs=qT[pair][hh * 64 : (hh + 1) * 64, q0 : q0 + QB],
                                        start=True, stop=True,
                                    )
                                pt_t = ptp.tile([P, 2 * QB], F32, tag="pt")
                                nc.scalar.activation(
                                    out=pt_t, in_=ps_st,
                                    func=mybir.ActivationFunctionType.Exp, scale=scale,
                                )
                                for hh in range(2):
                                    nc.tensor.matmul(
                                        pvTs[(j, hh)],
                                        lhsT=v_aug[kb][:, pair * 2 + hh, :],
                                        rhs=pt_t[:, hh * QB : (hh + 1) * QB],
                                        start=(kb == 0), stop=(kb == NT - 1),
                                    )
                        # epilogue: free PSUM accumulators via SBUF copy, then
                        # scale by 1/rowsum (row replicated via DRAM-bounce broadcast)
                        for j in range(2):
                            q0 = (qb2 * 2 + j) * QB
                            for hh in range(2):
                                pvsb = scp.tile([DH + 1, QB], F32, tag="pvsb")
                                nc.vector.tensor_copy(pvsb, pvTs[(j, hh)])
                                rec = smp.tile([1, QB], F32, tag="rec")
                                nc.vector.reciprocal(rec, pvsb[DH : DH + 1, :])
                                dsc = dpool.tile([1, QB], F32, tag="dsc")
                                nc.sync.dma_start(out=dsc, in_=rec)
                                rep = smp.tile([64, QB], F32, tag="repsb")
                                nc.sync.dma_start(
                                    out=rep,
                                    in_=bass.AP(tensor=dsc.tensor, offset=dsc.offset,
                                                ap=[[0, 64], list(dsc.ap[-1])]),
                                )
                                if hh == 0:
                                    nc.vector.tensor_mul(
                                        attnT[pair][0:64, q0 : q0 + QB],
                                        pvsb[0:DH, :], rep,
                                    )
                                else:
                                    sc_h = scp.tile([64, QB], F32, tag="sc")
                                    nc.vector.tensor_mul(sc_h, pvsb[0:DH, :], rep)
                                    nc.sync.dma_start(
                                        out=attnT[pair][64:128, q0 : q0 + QB], in_=sc_h
                                    )

            # ---------------- phase 4: out projection ----------------
            with tc.tile_pool(name="ps_out", bufs=4, space="PSUM") as pso, \
                 tc.tile_pool(name="sb_out", bufs=3) as sbo:
                for s in range(NT):
                    r0 = s * P
                    o_s = sbo.tile([P, D], F32, tag="osb")
                    for n in range(2):
                        ps_o = pso.tile([P, 512], F32, tag="out")
                        for c in range(4):
                            nc.tensor.matmul(
                                ps_o,
                                lhsT=attnT[c][:, r0 : r0 + P],
                                rhs=wo_s[:, c, n * 512 : (n + 1) * 512],
                                start=(c == 0), stop=(c == 3),
                            )
                        nc.vector.tensor_copy(
                            out=o_s[:, n * 512 : (n + 1) * 512], in_=ps_o
                        )
                    nc.scalar.dma_start(out=out_d[r0 : r0 + P, :], in_=o_s)
    nc.compile()
    return nc


def _rope_tables():
    inv = 1.0 / (BASE ** (np.arange(0, DH, 2, dtype=np.float32) / DH))
    t = np.arange(S, dtype=np.float32)
    freqs = t[:, None] * inv[None, :]  # [S, 32]
    cos_rep = np.tile(np.cos(freqs), (1, NH)).astype(np.float32)
    sin_rep = np.tile(np.sin(freqs), (1, NH)).astype(np.float32)
    return np.ascontiguousarray(cos_rep), np.ascontiguousarray(sin_rep)


def kernel(x, w_qkv, w_out, b_out, ln_gamma, ln_beta, _want_results=False, _trace=False):
    x = np.asarray(x, dtype=np.float32)
    w_qkv = np.asarray(w_qkv, dtype=np.float32)
    w_out = np.asarray(w_out, dtype=np.float32)
    b_out = np.asarray(b_out, dtype=np.float32)
    ln_gamma = np.asarray(ln_gamma, dtype=np.float32)
    ln_beta = np.asarray(ln_beta, dtype=np.float32)
    assert np.all(ln_beta == 0.0), "nonzero ln_beta not supported by this kernel"

    if "nc" not in _CACHE:
        _CACHE["nc"] = _build_nc()
    nc = _CACHE["nc"]

    wg = w_qkv * ln_gamma[:, None]  # fold gamma into the projection
    cos_rep, sin_rep = _rope_tables()
    bf = ml_dtypes.bfloat16

    in_maps = []
    for core in range(8):
        b, hg = core // HG, core % HG
        c0 = hg * IN
        in_maps.append({
            "x": np.ascontiguousarray(x[b]),
            "wq": np.ascontiguousarray(wg[:, c0 : c0 + IN]).astype(bf),
            "wk": np.ascontiguousarray(wg[:, D + c0 : D + c0 + IN]).astype(bf),
            "wv": np.ascontiguousarray(wg[:, 2 * D + c0 : 2 * D + c0 + IN]).astype(bf),
            "wo": np.ascontiguousarray(w_out[c0 : c0 + IN, :]),
            "cos_rep": cos_rep,
            "sin_rep": sin_rep,
        })

    res = run_bass_kernel_spmd(nc, in_maps, list(range(8)), trace=_trace)
    parts = [res.results[c]["out"] for c in range(8)]
    out = np.empty((B, S, D), dtype=np.float32)
    for b in range(B):
        out[b] = parts[2 * b] + parts[2 * b + 1] + b_out[None, :]
    if _want_results:
        return out, res
    return out


if __name__ == "__main__":
    rng = np.random.default_rng(0)
    inputs = {
        "x": rng.standard_normal((B, S, D), dtype=np.float32),
        "w_qkv": (rng.standard_normal((D, 3 * D), dtype=np.float32) * D ** -0.5),
        "w_out": (rng.standard_normal((D, D), dtype=np.float32) * D ** -0.5),
        "b_out": np.zeros(D, np.float32),
        "ln_gamma": np.ones(D, np.float32),
        "ln_beta": np.zeros(D, np.float32),
    }
    out = kernel(**inputs)
    print("ok", out.shape, out.dtype)



# revision 25
# speedup vs baseline: 2.3701x; 2.3701x over previous
"""Trainium2 Bass kernel for nn_Attention_RoPE (LN -> QKV -> RoPE -> attention -> out-proj).

Sharding: 8 cores = 4 batches x 2 head-groups (8 heads each).
Each core computes a partial out-projection [S, D] for its (batch, head-group);
host sums the two partials per batch and adds b_out.

Per-core pipeline (single Bass program, SPMD over 8 cores):
  phase 1+2 (fused, per 128-row seq tile), engine-balanced:
    LN (bn_stats/bn_aggr on DVE) -> PE transpose to xnT (copies on Pool)
    -> QKV matmuls (bf16) -> RoPE on q (DVE) and k (Pool) from PSUM
    -> PE transpose rot -> qT/kT [feat, seq] (copies on Pool)
    -> v copied to SBUF bf16 with an appended ones column (DVE)
    x/out DMAs ride SP; cos/sin tables preloaded once (bf16, persistent).
  phase 3 (per q-block of 512, per head-pair):
    S^T = K @ Q^T (row-packed, 2 heads per 128 psum partitions)
    exp on ScalarE over [128, 1024] tiles -> bf16 SBUF (kept live for all
    4 q-subblocks)
    PV with OUT = [q, d]: lhsT = exp(S^T) 128q-slice, rhs = [v | ones] bf16
    (65 moving cols only -> half the PE cost of the [d, q] orientation).
    Both heads' PV chains share one PSUM bank: head 1 rides head 0's
    pending-zero region with start=False (skip_group_check).
    epilogue: per-partition reciprocal of the ones-column sum + scale (DVE),
    DMA-transpose (SP XBAR) into attnT [feat, seq]
  phase 4: out-projection (bf16), interleaved one seq-tile per pair slot of
    the NEXT q-block so PE work fills ACT-bound slack.
"""

import numpy as np
import sys

sys.path.insert(0, "/opt/trn_rl_repo")

import ml_dtypes

import concourse.bass as bass
from concourse import bacc
import concourse.mybir as mybir
import concourse.tile as tile
from concourse.bass_utils import run_bass_kernel_spmd

# Problem constants (hardcoded per contract)
B, S, D = 4, 2048, 1024
H, DH = 16, 64
HG = 2              # head groups (tensor-parallel dim)
NH = H // HG        # heads per core = 8
IN = NH * DH        # per-core inner dim = 512
P = 128
NT = S // P         # 16 seq tiles
NCK = D // P        # 8 contraction chunks
NPAIR = NH // 2     # 4 head pairs
QB = 512            # query block in phase 3
EPS = 1e-5
BASE = 10000.0

F32 = mybir.dt.float32
BF16 = mybir.dt.bfloat16
I16 = mybir.dt.int16

# fast-exp (PWL exp2 via bf16 bit pattern): bf16(exp(x)) ~= bitcast_bf16(
#   int16(round(x * 128*log2e + (127*128 - C)))).  C tunes the PWL bias.
FE_A = 128.0 * 1.4426950408889634
FE_C = 6.0
FE_B = 127.0 * 128.0 - FE_C
# which kb slots of each 16-step chain use DVE fast-exp (rest: exact ACT exp)
FAST_KBS = frozenset({2, 5, 8, 11, 14, 15})

_CACHE = {}


def _build_nc():
    nc = bacc.Bacc(None, target_bir_lowering=False, debug=False)

    x_d = nc.declare_dram_parameter("x", [S, D], F32, isOutput=False)
    wq_d = nc.declare_dram_parameter("wq", [D, IN], BF16, isOutput=False)
    wk_d = nc.declare_dram_parameter("wk", [D, IN], BF16, isOutput=False)
    wv_d = nc.declare_dram_parameter("wv", [D, IN], BF16, isOutput=False)
    wo_d = nc.declare_dram_parameter("wo", [IN, D], BF16, isOutput=False)
    cos_d = nc.declare_dram_parameter("cos_rep", [S, DH], BF16, isOutput=False)
    sin_d = nc.declare_dram_parameter("sin_rep", [S, DH], BF16, isOutput=False)
    out_d = nc.declare_dram_parameter("out", [S, D], F32, isOutput=True)

    with tile.TileContext(nc) as tc:
        with tc.tile_pool(name="persist", bufs=1) as pers:
            eps_t = pers.tile([P, 1], F32)
            nc.vector.memset(eps_t, EPS)

            # weights resident in SBUF (declared now, loaded after the first
            # x prefetches below so tile 0 isn't stuck behind them on SP)
            wq_s = pers.tile([P, NCK, IN], BF16, tag="wq")
            wk_s = pers.tile([P, NCK, IN], BF16, tag="wk")
            wv_s = pers.tile([P, NCK, IN], BF16, tag="wv")
            wo_s = pers.tile([P, NPAIR, D], BF16, tag="wo")

            # doubled cos/sin tables ([c|c], [s|s]), one head's worth (bf16);
            # broadcast across heads via 0-stride APs at use sites
            cos_all = pers.tile([P, NT, DH], BF16, tag="cos")
            sin_all = pers.tile([P, NT, DH], BF16, tag="sin")

            # persistent activations: merged [feat-in-block, pair, seq] layouts
            qT_all = pers.tile([P, NPAIR, S], BF16, tag="qT", name="qT")
            kT_all = pers.tile([P, NPAIR, S], BF16, tag="kT", name="kT")
            # v (bf16) with ones column appended per head: [kpos, ktile, head, 65]
            v_aug = pers.tile([P, NT, NH, DH + 1], BF16, tag="vaug", name="vaug")
            nc.vector.memset(v_aug[:, :, :, DH : DH + 1], 1.0)
            attnT_all = pers.tile([P, NPAIR, S], BF16, tag="aT", name="aT")

            scale = 1.0 / np.sqrt(DH)
            ride_pts = []
            # phase-3 SBUF pools opened early: the ride (below) emits exp tiles
            # for (j=0, pair=0) into pt_pool during phase 1+2
            with tc.tile_pool(name="pt_pool", bufs=17) as ptp, \
                 tc.tile_pool(name="ep_pool", bufs=6) as epp, \
                 tc.tile_pool(name="o_pool", bufs=2) as osp:
              # ---------------- phase 1 + 2 ----------------
              with tc.tile_pool(name="ph12", bufs=3) as tp, \
                 tc.tile_pool(name="ph12s", bufs=3) as sp, \
                 tc.tile_pool(name="ps_ride", bufs=1, space="PSUM") as pstr, \
                 tc.tile_pool(name="ps_proj", bufs=2, space="PSUM") as ppr:

                def emit_ride(kb):
                    ps_st = pstr.tile([P, 2, QB], F32, tag="str")
                    for hh in range(2):
                        nc.tensor.matmul(
                            ps_st[:, hh, :],
                            lhsT=kT_all[hh * 64 : (hh + 1) * 64, 0, kb * P : (kb + 1) * P],
                            rhs=qT_all[hh * 64 : (hh + 1) * 64, 0, 0:QB],
                            start=True, stop=True,
                        )
                    pt_t = ptp.tile([P, 2, QB], BF16, tag="pt")
                    nc.scalar.activation(
                        out=pt_t, in_=ps_st,
                        func=mybir.ActivationFunctionType.Exp, scale=scale,
                    )
                    ride_pts.append(pt_t)
                x_tiles = {}
                x_tiles[0] = tp.tile([P, D], F32, tag="x", name="xpre0")
                nc.sync.dma_start(out=x_tiles[0], in_=x_d[0:P, :])
                # wq rides SP right behind x0; wk/wo/wv are injected just-in-time
                # inside tile 0 below so no queue blocks tile 0's LN chain
                nc.sync.dma_start(out=wq_s, in_=wq_d.rearrange("(c p) n -> p c n", p=P))
                x_tiles[1] = tp.tile([P, D], F32, tag="x", name="xpre1")
                nc.sync.dma_start(out=x_tiles[1], in_=x_d[P : 2 * P, :])
                for w_s, w_d in ((wk_s, wk_d), (wv_s, wv_d), (wo_s, wo_d)):
                    nc.scalar.dma_start(
                        out=w_s, in_=w_d.rearrange("(c p) n -> p c n", p=P)
                    )
                for s in range(NT):
                    r0 = s * P
                    if s in x_tiles:
                        x_s = x_tiles.pop(s)
                    else:
                        x_s = tp.tile([P, D], F32, tag="x")
                        nc.sync.dma_start(out=x_s, in_=x_d[r0 : r0 + P, :])

                    nc.gpsimd.dma_start(out=cos_all[:, s], in_=cos_d[r0 : r0 + P, :])
                    nc.gpsimd.dma_start(out=sin_all[:, s], in_=sin_d[r0 : r0 + P, :])
                    stats = sp.tile([P, 2, 6], F32, tag="stats")
                    for i in range(2):
                        nc.vector.bn_stats(
                            out=stats[:, i, :], in_=x_s[:, i * 512 : (i + 1) * 512]
                        )
                    mv = sp.tile([P, 2], F32, tag="mv")
                    nc.vector.bn_aggr(out=mv, in_=stats)
                    std = sp.tile([P, 1], F32, tag="std")
                    nc.scalar.activation(
                        out=std, in_=mv[:, 1:2],
                        func=mybir.ActivationFunctionType.Sqrt, bias=eps_t,
                    )
                    rstd = sp.tile([P, 1], F32, tag="rstd")
                    nc.vector.reciprocal(out=rstd, in_=std)

                    xn_s = tp.tile([P, D], BF16, tag="xn")
                    nc.gpsimd.tensor_scalar(
                        out=xn_s, in0=x_s, scalar1=mv[:, 0:1], scalar2=rstd,
                        op0=mybir.AluOpType.subtract, op1=mybir.AluOpType.mult,
                    )

                    # transpose xn -> xnT blocks [dim-in-chunk, chunk, seq] via XBAR
                    xnT_s = tp.tile([P, NCK, P], BF16, tag="xnT")
                    nc.sync.dma_start_transpose(out=xnT_s, in_=xn_s)

                    # projections: q, k, v  [128 seq, 512 feat]
                    ps_q = ppr.tile([P, IN], F32, tag="pq")
                    ps_k = ppr.tile([P, IN], F32, tag="pk")
                    ps_v = ppr.tile([P, IN], F32, tag="pv")
                    for ps, w_s in ((ps_q, wq_s), (ps_k, wk_s), (ps_v, wv_s)):
                        for c in range(NCK):
                            nc.tensor.matmul(
                                ps, lhsT=xnT_s[:, c, :], rhs=w_s[:, c, :],
                                start=(c == 0), stop=(c == NCK - 1),
                            )

                    # v -> SBUF bf16 (ones column preset once above)
                    nc.vector.tensor_copy(
                        out=v_aug[:, s, :, 0:DH],
                        in_=ps_v.rearrange("p (h d) -> p h d", h=NH),
                    )

                    # RoPE: q on DVE, k on Pool; 4-op form via doubled tables:
                    # P1 = [x1*c | x2*c], P2 = [x1*s | x2*s]
                    # out = [P1.lo - P2.hi | P2.lo + P1.hi]
                    cos_s = cos_all[:, s].unsqueeze(1).broadcast_to((P, NH, DH))
                    sin_s = sin_all[:, s].unsqueeze(1).broadcast_to((P, NH, DH))
                    # gpsimd cannot touch PSUM on HW: drain k to SBUF via ACT first
                    k_sb = sp.tile([P, NH, DH], BF16, tag="ksb")
                    nc.vector.tensor_copy(out=k_sb, in_=ps_k.rearrange("p (h d) -> p h d", h=NH))
                    rots = {}
                    for name, src_, eng in (
                        ("q", ps_q.rearrange("p (h d) -> p h d", h=NH), nc.vector),
                        ("k", k_sb, nc.gpsimd),
                    ):
                        rot = tp.tile([P, NH, DH], BF16, tag=f"rot{name}")
                        p1 = sp.tile([P, NH, DH], BF16, tag=f"p1{name}")
                        p2 = sp.tile([P, NH, DH], BF16, tag=f"p2{name}")
                        eng.tensor_mul(p1, src_, cos_s)
                        eng.tensor_mul(p2, src_, sin_s)
                        eng.tensor_sub(rot[:, :, 0:32], p1[:, :, 0:32], p2[:, :, 32:64])
                        eng.tensor_add(rot[:, :, 32:64], p2[:, :, 0:32], p1[:, :, 32:64])
                        rots[name] = rot
                    if s >= 4:
                        emit_ride(s - 4)
                    nc.sync.dma_start_transpose(
                        out=qT_all[:, :, r0 : r0 + P],
                        in_=rots["q"].rearrange("p h d -> p (h d)"),
                    )
                    nc.sync.dma_start_transpose(
                        out=kT_all[:, :, r0 : r0 + P],
                        in_=rots["k"].rearrange("p h d -> p (h d)"),
                    )

                for kb in range(NT - 4, NT):
                    emit_ride(kb)

              # ---------------- phase 3 + 4: attention + out-projection ----------------
              with tc.tile_pool(name="ps_st", bufs=2, space="PSUM") as pst, \
                 tc.tile_pool(name="ps_pv", bufs=2, space="PSUM") as ppv, \
                 tc.tile_pool(name="ps_out", bufs=2, space="PSUM") as pso:

                def emit_outproj_tile(r0):
                    o_s = osp.tile([P, D], F32, tag="osb")
                    for n in range(2):
                        ps_o = pso.tile([P, 512], F32, tag="out")
                        for c in range(NPAIR):
                            nc.tensor.matmul(
                                ps_o,
                                lhsT=attnT_all[:, c, r0 : r0 + P],
                                rhs=wo_s[:, c, n * 512 : (n + 1) * 512],
                                start=(c == 0), stop=(c == NPAIR - 1),
                            )
                        nc.vector.tensor_copy(
                            out=o_s[:, n * 512 : (n + 1) * 512], in_=ps_o
                        )
                    nc.sync.dma_start(out=out_d[r0 : r0 + P, :], in_=o_s)

                for j in range(4):              # q block of 512
                    q0 = j * QB
                    for pair in range(NPAIR):
                        if j == 0 and pair == 0:
                            # scores+exp already computed by the phase-1+2 ride
                            pts = ride_pts
                            pv = ppv.tile([P, 2, DH + 1], F32, tag="pv", name="pv")
                            for kb in range(NT):
                                for hh in range(2):
                                    nc.tensor.matmul(
                                        pv[:, hh, :],
                                        lhsT=pts[kb][:, hh, 0:P],
                                        rhs=v_aug[:, kb, pair * 2 + hh, :],
                                        start=(kb == 0 and hh == 0), stop=(kb == NT - 1),
                                        skip_group_check=True,
                                    )
                        else:
                          pts = []
                          pv = None
                          for kb in range(NT):
                            ps_st = pst.tile([P, 2, QB], F32, tag="st")
                            for hh in range(2):
                                nc.tensor.matmul(
                                    ps_st[:, hh, :],
                                    lhsT=kT_all[hh * 64 : (hh + 1) * 64, pair, kb * P : (kb + 1) * P],
                                    rhs=qT_all[hh * 64 : (hh + 1) * 64, pair, q0 : q0 + QB],
                                    start=True, stop=True,
                                )
                            pt_t = ptp.tile([P, 2, QB], BF16, tag="pt")
                            if kb in FAST_KBS:
                                nc.vector.tensor_scalar(
                                    out=pt_t.bitcast(I16), in0=ps_st,
                                    scalar1=float(FE_A * scale), scalar2=float(FE_B),
                                    op0=mybir.AluOpType.mult, op1=mybir.AluOpType.add,
                                )
                            else:
                                nc.scalar.activation(
                                    out=pt_t, in_=ps_st,
                                    func=mybir.ActivationFunctionType.Exp, scale=scale,
                                )
                            pts.append(pt_t)
                            # PV for q-subblock 0 rides along with the kb loop
                            if kb == 0:
                                pv = ppv.tile([P, 2, DH + 1], F32, tag="pv", name="pv")
                            for hh in range(2):
                                nc.tensor.matmul(
                                    pv[:, hh, :],
                                    lhsT=pt_t[:, hh, 0:P],
                                    rhs=v_aug[:, kb, pair * 2 + hh, :],
                                    start=(kb == 0 and hh == 0), stop=(kb == NT - 1),
                                    skip_group_check=True,
                                )
                        for qs in range(4):
                            if qs > 0:
                                pv = ppv.tile([P, 2, DH + 1], F32, tag="pv", name="pv")
                                for kb in range(NT):
                                    for hh in range(2):
                                        nc.tensor.matmul(
                                            pv[:, hh, :],
                                            lhsT=pts[kb][:, hh, qs * P : (qs + 1) * P],
                                            rhs=v_aug[:, kb, pair * 2 + hh, :],
                                            start=(kb == 0 and hh == 0), stop=(kb == NT - 1),
                                            skip_group_check=True,
                                        )
                            # epilogue: normalize by the ones-column sum, transpose out
                            asb = epp.tile([P, 2, DH], BF16, tag="asb")
                            for hh in range(2):
                                rec = epp.tile([P, 1], F32, tag=f"rec{hh}")
                                nc.vector.reciprocal(out=rec, in_=pv[:, hh, DH : DH + 1])
                                nc.vector.tensor_scalar_mul(
                                    out=asb[:, hh, :], in0=pv[:, hh, 0:DH],
                                    scalar1=rec,
                                )
                            nc.sync.dma_start_transpose(
                                out=attnT_all[:, pair, q0 + qs * P : q0 + (qs + 1) * P],
                                in_=asb.rearrange("p a b -> p (a b)"),
                            )
                            if j == 3 and pair == NPAIR - 1:
                                emit_outproj_tile(q0 + qs * P)
                        # interleave one out-proj seq tile of the PREVIOUS q block
                        if j > 0:
                            emit_outproj_tile((j - 1) * QB + pair * P)
    nc.compile()
    return nc


def _rope_tables():
    inv = 1.0 / (BASE ** (np.arange(0, DH, 2, dtype=np.float32) / DH))
    t = np.arange(S, dtype=np.float32)
    freqs = t[:, None] * inv[None, :]  # [S, 32]
    bf = ml_dtypes.bfloat16
    # doubled per head: [c|c] and [s|s] over the 64-dim head
    cos_rep = np.concatenate([np.cos(freqs), np.cos(freqs)], axis=1).astype(bf)  # [S, 64]
    sin_rep = np.concatenate([np.sin(freqs), np.sin(freqs)], axis=1).astype(bf)
    return np.ascontiguousarray(cos_rep), np.ascontiguousarray(sin_rep)


def kernel(x, w_qkv, w_out, b_out, ln_gamma, ln_beta, _want_results=False, _trace=False):
    x = np.asarray(x, dtype=np.float32)
    w_qkv = np.asarray(w_qkv, dtype=np.float32)
    w_out = np.asarray(w_out, dtype=np.float32)
    b_out = np.asarray(b_out, dtype=np.float32)
    ln_gamma = np.asarray(ln_gamma, dtype=np.float32)
    ln_beta = np.asarray(ln_beta, dtype=np.float32)
    assert np.all(ln_beta == 0.0), "nonzero ln_beta not supported by this kernel"

    if "nc" not in _CACHE:
        _CACHE["nc"] = _build_nc()
    nc = _CACHE["nc"]

    wg = w_qkv * ln_gamma[:, None]  # fold gamma into the projection
    cos_rep, sin_rep = _rope_tables()
    bf = ml_dtypes.bfloat16

    in_maps = []
    for core in range(8):
        b, hg = core // HG, core % HG
        c0 = hg * IN
        in_maps.append({
            "x": np.ascontiguousarray(x[b]),
            "wq": np.ascontiguousarray(wg[:, c0 : c0 + IN]).astype(bf),
            "wk": np.ascontiguousarray(wg[:, D + c0 : D + c0 + IN]).astype(bf),
            "wv": np.ascontiguousarray(wg[:, 2 * D + c0 : 2 * D + c0 + IN]).astype(bf),
            "wo": np.ascontiguousarray(w_out[c0 : c0 + IN, :]).astype(bf),
            "cos_rep": cos_rep,
            "sin_rep": sin_rep,
        })

    res = run_bass_kernel_spmd(nc, in_maps, list(range(8)), trace=_trace)
    parts = [res.results[c]["out"] for c in range(8)]
    out = np.empty((B, S, D), dtype=np.float32)
    for b in range(B):
        out[b] = parts[2 * b] + parts[2 * b + 1] + b_out[None, :]
    if _want_results:
        return out, res
    return out


if __name__ == "__main__":
    rng = np.random.default_rng(0)
    inputs = {
        "x": rng.standard_normal((B, S, D), dtype=np.float32),
        "w_qkv": (rng.standard_normal((D, 3 * D), dtype=np.float32) * D ** -0.5),
        "w_out": (rng.standard_normal((D, D), dtype=np.float32) * D ** -0.5),
        "b_out": np.zeros(D, np.float32),
        "ln_gamma": np.ones(D, np.float32),
        "ln_beta": np.zeros(D, np.float32),
    }
    out = kernel(**inputs)
    print("ok", out.shape, out.dtype)


# revision 33
# speedup vs baseline: 2.5226x; 1.0643x over previous
"""Trainium2 Bass kernel for nn_Attention_RoPE (LN -> QKV -> RoPE -> attention -> out-proj).

Sharding: 8 cores = 4 batches x 2 head-groups (8 heads each).
Each core computes a partial out-projection [S, D] for its (batch, head-group);
host sums the two partials per batch and adds b_out.

Per-core pipeline (single Bass program, SPMD over 8 cores):
  phase 1+2 (fused, per 128-row seq tile), engine-balanced:
    LN (bn_stats/bn_aggr on DVE) -> PE transpose to xnT (copies on Pool)
    -> QKV matmuls (bf16) -> RoPE on q (DVE) and k (Pool) from PSUM
    -> PE transpose rot -> qT/kT [feat, seq] (copies on Pool)
    -> v copied to SBUF bf16 with an appended ones column (DVE)
    x/out DMAs ride SP; cos/sin tables preloaded once (bf16, persistent).
  phase 3 (per q-block of 512, per head-pair):
    S^T = K @ Q^T (row-packed, 2 heads per 128 psum partitions)
    exp on ScalarE over [128, 1024] tiles -> bf16 SBUF (kept live for all
    4 q-subblocks)
    PV with OUT = [q, d]: lhsT = exp(S^T) 128q-slice, rhs = [v | ones] bf16
    (65 moving cols only -> half the PE cost of the [d, q] orientation).
    Both heads' PV chains share one PSUM bank: head 1 rides head 0's
    pending-zero region with start=False (skip_group_check).
    epilogue: per-partition reciprocal of the ones-column sum + scale (DVE),
    DMA-transpose (SP XBAR) into attnT [feat, seq]
  phase 4: out-projection (bf16), interleaved one seq-tile per pair slot of
    the NEXT q-block so PE work fills ACT-bound slack.
"""

import numpy as np
import sys

sys.path.insert(0, "/opt/trn_rl_repo")

import ml_dtypes

import concourse.bass as bass
from concourse import bacc
import concourse.mybir as mybir
import concourse.tile as tile
from concourse.bass_utils import run_bass_kernel_spmd

# Problem constants (hardcoded per contract)
B, S, D = 4, 2048, 1024
H, DH = 16, 64
HG = 2              # head groups (tensor-parallel dim)
NH = H // HG        # heads per core = 8
IN = NH * DH        # per-core inner dim = 512
P = 128
NT = S // P         # 16 seq tiles
NCK = D // P        # 8 contraction chunks
NPAIR = NH // 2     # 4 head pairs
QB = 512            # query block in phase 3
EPS = 1e-5
BASE = 10000.0

F32 = mybir.dt.float32
BF16 = mybir.dt.bfloat16
I16 = mybir.dt.int16

# fast-exp (PWL exp2 via bf16 bit pattern): bf16(exp(x)) ~= bitcast_bf16(
#   int16(round(x * 128*log2e + (127*128 - C)))).  C tunes the PWL bias.
FE_A = 128.0 * 1.4426950408889634
FE_C = 6.0
FE_B = 127.0 * 128.0 - FE_C
# which kb slots of each 16-step chain use DVE fast-exp (rest: exact ACT exp)
FAST_KBS = frozenset({2, 5, 8, 11, 14, 15})

_CACHE = {}


def _build_nc():
    nc = bacc.Bacc(None, target_bir_lowering=False, debug=False)

    x_d = nc.declare_dram_parameter("x", [S, D], F32, isOutput=False)
    wq_d = nc.declare_dram_parameter("wq", [D, IN], BF16, isOutput=False)
    wk_d = nc.declare_dram_parameter("wk", [D, IN], BF16, isOutput=False)
    wv_d = nc.declare_dram_parameter("wv", [D, IN], BF16, isOutput=False)
    wo_d = nc.declare_dram_parameter("wo", [IN, D], BF16, isOutput=False)
    cos_d = nc.declare_dram_parameter("cos_rep", [S, DH], BF16, isOutput=False)
    sin_d = nc.declare_dram_parameter("sin_rep", [S, DH], BF16, isOutput=False)
    out_d = nc.declare_dram_parameter("out", [S, D], F32, isOutput=True)

    with tile.TileContext(nc) as tc:
        with tc.tile_pool(name="persist", bufs=1) as pers:
            eps_t = pers.tile([P, 1], F32)
            nc.vector.memset(eps_t, EPS)

            # weights resident in SBUF (declared now, loaded after the first
            # x prefetches below so tile 0 isn't stuck behind them on SP)
            wq_s = pers.tile([P, NCK, IN], BF16, tag="wq")
            wk_s = pers.tile([P, NCK, IN], BF16, tag="wk")
            wv_s = pers.tile([P, NCK, IN], BF16, tag="wv")
            wo_s = pers.tile([P, NPAIR, D], BF16, tag="wo")

            # doubled cos/sin tables ([c|c], [s|s]), one head's worth (bf16);
            # broadcast across heads via 0-stride APs at use sites
            cos_all = pers.tile([P, NT, DH], BF16, tag="cos")
            sin_all = pers.tile([P, NT, DH], BF16, tag="sin")

            # persistent activations: merged [feat-in-block, pair, seq] layouts
            qT_all = pers.tile([P, NPAIR, S], BF16, tag="qT", name="qT")
            kT_all = pers.tile([P, NPAIR, S], BF16, tag="kT", name="kT")
            # v (bf16) with ones column appended per head: [kpos, ktile, head, 65]
            v_aug = pers.tile([P, NT, NH, DH + 1], BF16, tag="vaug", name="vaug")
            nc.vector.memset(v_aug[:, :, :, DH : DH + 1], 1.0)
            attnT_all = pers.tile([P, NPAIR, S], BF16, tag="aT", name="aT")

            scale = 1.0 / np.sqrt(DH)
            ride_pts = []
            # phase-3 SBUF pools opened early: the ride (below) emits exp tiles
            # for (j=0, pair=0) into pt_pool during phase 1+2
            with tc.tile_pool(name="pt_pool", bufs=17) as ptp, \
                 tc.tile_pool(name="ep_pool", bufs=6) as epp, \
                 tc.tile_pool(name="o_pool", bufs=2) as osp:
              # ---------------- phase 1 + 2 ----------------
              with tc.tile_pool(name="ph12", bufs=3) as tp, \
                 tc.tile_pool(name="ph12s", bufs=3) as sp, \
                 tc.tile_pool(name="ps_ride", bufs=1, space="PSUM") as pstr, \
                 tc.tile_pool(name="ps_proj", bufs=2, space="PSUM") as ppr:

                def emit_ride(kb):
                    ps_st = pstr.tile([P, 2, QB], F32, tag="str")
                    for hh in range(2):
                        nc.tensor.matmul(
                            ps_st[:, hh, :],
                            lhsT=kT_all[hh * 64 : (hh + 1) * 64, 0, kb * P : (kb + 1) * P],
                            rhs=qT_all[hh * 64 : (hh + 1) * 64, 0, 0:QB],
                            start=True, stop=True,
                        )
                    pt_t = ptp.tile([P, 2, QB], BF16, tag="pt")
                    nc.scalar.activation(
                        out=pt_t, in_=ps_st,
                        func=mybir.ActivationFunctionType.Exp, scale=scale,
                    )
                    ride_pts.append(pt_t)
                x_tiles = {}
                x_tiles[0] = tp.tile([P, D], F32, tag="x", name="xpre0")
                nc.sync.dma_start(out=x_tiles[0], in_=x_d[0:P, :])
                # wq rides SP right behind x0; wk/wo/wv are injected just-in-time
                # inside tile 0 below so no queue blocks tile 0's LN chain
                nc.sync.dma_start(out=wq_s, in_=wq_d.rearrange("(c p) n -> p c n", p=P))
                x_tiles[1] = tp.tile([P, D], F32, tag="x", name="xpre1")
                nc.sync.dma_start(out=x_tiles[1], in_=x_d[P : 2 * P, :])
                for w_s, w_d in ((wk_s, wk_d), (wv_s, wv_d)):
                    nc.scalar.dma_start(
                        out=w_s[:, 0:4, :],
                        in_=w_d.rearrange("(c p) n -> p c n", p=P)[:, 0:4, :],
                    )
                for s in range(NT):
                    r0 = s * P
                    if s in x_tiles:
                        x_s = x_tiles.pop(s)
                    else:
                        x_s = tp.tile([P, D], F32, tag="x")
                        nc.sync.dma_start(out=x_s, in_=x_d[r0 : r0 + P, :])

                    nc.gpsimd.dma_start(out=cos_all[:, s], in_=cos_d[r0 : r0 + P, :])
                    nc.gpsimd.dma_start(out=sin_all[:, s], in_=sin_d[r0 : r0 + P, :])
                    stats = sp.tile([P, 2, 6], F32, tag="stats")
                    for i in range(2):
                        nc.vector.bn_stats(
                            out=stats[:, i, :], in_=x_s[:, i * 512 : (i + 1) * 512]
                        )
                    mv = sp.tile([P, 2], F32, tag="mv")
                    nc.vector.bn_aggr(out=mv, in_=stats)
                    std = sp.tile([P, 1], F32, tag="std")
                    nc.scalar.activation(
                        out=std, in_=mv[:, 1:2],
                        func=mybir.ActivationFunctionType.Sqrt, bias=eps_t,
                    )
                    rstd = sp.tile([P, 1], F32, tag="rstd")
                    nc.vector.reciprocal(out=rstd, in_=std)

                    xn_s = tp.tile([P, D], BF16, tag="xn")
                    nc.gpsimd.tensor_scalar(
                        out=xn_s, in0=x_s, scalar1=mv[:, 0:1], scalar2=rstd,
                        op0=mybir.AluOpType.subtract, op1=mybir.AluOpType.mult,
                    )

                    # transpose xn -> xnT blocks [dim-in-chunk, chunk, seq] via XBAR
                    xnT_s = tp.tile([P, NCK, P], BF16, tag="xnT")
                    nc.sync.dma_start_transpose(out=xnT_s, in_=xn_s)
                    if s == 0:
                        nc.sync.dma_start(
                            out=wk_s[:, 4:NCK, :],
                            in_=wk_d.rearrange("(c p) n -> p c n", p=P)[:, 4:NCK, :],
                        )

                    # projections: q, k, v  [128 seq, 512 feat]
                    ps_q = ppr.tile([P, IN], F32, tag="pq")
                    ps_k = ppr.tile([P, IN], F32, tag="pk")
                    ps_v = ppr.tile([P, IN], F32, tag="pv")
                    for ps, w_s in ((ps_q, wq_s), (ps_k, wk_s), (ps_v, wv_s)):
                        for c in range(NCK):
                            nc.tensor.matmul(
                                ps, lhsT=xnT_s[:, c, :], rhs=w_s[:, c, :],
                                start=(c == 0), stop=(c == NCK - 1),
                            )

                    # v -> SBUF bf16 (ones column preset once above)
                    nc.vector.tensor_copy(
                        out=v_aug[:, s, :, 0:DH],
                        in_=ps_v.rearrange("p (h d) -> p h d", h=NH),
                    )

                    # RoPE: q on DVE, k on Pool; 4-op form via doubled tables:
                    # P1 = [x1*c | x2*c], P2 = [x1*s | x2*s]
                    # out = [P1.lo - P2.hi | P2.lo + P1.hi]
                    cos_s = cos_all[:, s].unsqueeze(1).broadcast_to((P, NH, DH))
                    sin_s = sin_all[:, s].unsqueeze(1).broadcast_to((P, NH, DH))
                    # gpsimd cannot touch PSUM on HW: drain k to SBUF via ACT first
                    k_sb = sp.tile([P, NH, DH], BF16, tag="ksb")
                    nc.vector.tensor_copy(out=k_sb, in_=ps_k.rearrange("p (h d) -> p h d", h=NH))
                    rots = {}
                    for name, src_, eng in (
                        ("q", ps_q.rearrange("p (h d) -> p h d", h=NH), nc.vector),
                        ("k", k_sb, nc.gpsimd),
                    ):
                        rot = tp.tile([P, NH, DH], BF16, tag=f"rot{name}")
                        p1 = sp.tile([P, NH, DH], BF16, tag=f"p1{name}")
                        p2 = sp.tile([P, NH, DH], BF16, tag=f"p2{name}")
                        eng.tensor_mul(p1, src_, cos_s)
                        eng.tensor_mul(p2, src_, sin_s)
                        eng.tensor_sub(rot[:, :, 0:32], p1[:, :, 0:32], p2[:, :, 32:64])
                        eng.tensor_add(rot[:, :, 32:64], p2[:, :, 0:32], p1[:, :, 32:64])
                        rots[name] = rot
                    if s >= 4:
                        emit_ride(s - 4)
                    nc.sync.dma_start_transpose(
                        out=qT_all[:, :, r0 : r0 + P],
                        in_=rots["q"].rearrange("p h d -> p (h d)"),
                    )
                    nc.sync.dma_start_transpose(
                        out=kT_all[:, :, r0 : r0 + P],
                        in_=rots["k"].rearrange("p h d -> p (h d)"),
                    )


                for kb in range(NT - 4, NT):
                    emit_ride(kb)

              # ---------------- phase 3 + 4: attention + out-projection ----------------
              with tc.tile_pool(name="ps_st", bufs=2, space="PSUM") as pst, \
                 tc.tile_pool(name="ps_pv", bufs=2, space="PSUM") as ppv, \
                 tc.tile_pool(name="ps_out", bufs=2, space="PSUM") as pso:

                def emit_outproj_tile(r0):
                    o_s = osp.tile([P, D], F32, tag="osb")
                    for n in range(2):
                        ps_o = pso.tile([P, 512], F32, tag="out")
                        for c in range(NPAIR):
                            nc.tensor.matmul(
                                ps_o,
                                lhsT=attnT_all[:, c, r0 : r0 + P],
                                rhs=wo_s[:, c, n * 512 : (n + 1) * 512],
                                start=(c == 0), stop=(c == NPAIR - 1),
                            )
                        nc.vector.tensor_copy(
                            out=o_s[:, n * 512 : (n + 1) * 512], in_=ps_o
                        )
                    nc.sync.dma_start(out=out_d[r0 : r0 + P, :], in_=o_s)

                for j in range(4):              # q block of 512
                    q0 = j * QB
                    for pair in range(NPAIR):
                        # out-proj seq tile of the PREVIOUS q block, emitted
                        # ahead of this pair's chain so it never sits between
                        # the pair's last exp and the next pair's scores
                        if j > 0:
                            emit_outproj_tile((j - 1) * QB + pair * P)
                        if j == 0 and pair == 0:
                            # scores+exp already computed by the phase-1+2 ride
                            pts = ride_pts
                            pv = ppv.tile([P, 2, DH + 1], F32, tag="pv", name="pv")
                            for kb in range(NT):
                                for hh in range(2):
                                    nc.tensor.matmul(
                                        pv[:, hh, :],
                                        lhsT=pts[kb][:, hh, 0:P],
                                        rhs=v_aug[:, kb, pair * 2 + hh, :],
                                        start=(kb == 0 and hh == 0), stop=(kb == NT - 1),
                                        skip_group_check=True,
                                    )
                        else:
                          pts = []
                          pv = None
                          for kb in range(NT):
                            ps_st = pst.tile([P, 2, QB], F32, tag="st")
                            for hh in range(2):
                                nc.tensor.matmul(
                                    ps_st[:, hh, :],
                                    lhsT=kT_all[hh * 64 : (hh + 1) * 64, pair, kb * P : (kb + 1) * P],
                                    rhs=qT_all[hh * 64 : (hh + 1) * 64, pair, q0 : q0 + QB],
                                    start=True, stop=True,
                                )
                            pt_t = ptp.tile([P, 2, QB], BF16, tag="pt")
                            if kb in FAST_KBS:
                                nc.vector.tensor_scalar(
                                    out=pt_t.bitcast(I16), in0=ps_st,
                                    scalar1=float(FE_A * scale), scalar2=float(FE_B),
                                    op0=mybir.AluOpType.mult, op1=mybir.AluOpType.add,
                                )
                            else:
                                nc.scalar.activation(
                                    out=pt_t, in_=ps_st,
                                    func=mybir.ActivationFunctionType.Exp, scale=scale,
                                )
                            pts.append(pt_t)
                            # PV for q-subblock 0 rides along with the kb loop
                            if kb == 0:
                                pv = ppv.tile([P, 2, DH + 1], F32, tag="pv", name="pv")
                            for hh in range(2):
                                nc.tensor.matmul(
                                    pv[:, hh, :],
                                    lhsT=pt_t[:, hh, 0:P],
                                    rhs=v_aug[:, kb, pair * 2 + hh, :],
                                    start=(kb == 0 and hh == 0), stop=(kb == NT - 1),
                                    skip_group_check=True,
                                )
                        for qs in range(4):
                            if qs > 0:
                                pv = ppv.tile([P, 2, DH + 1], F32, tag="pv", name="pv")
                                for kb in range(NT):
                                    for hh in range(2):
                                        nc.tensor.matmul(
                                            pv[:, hh, :],
                                            lhsT=pts[kb][:, hh, qs * P : (qs + 1) * P],
                                            rhs=v_aug[:, kb, pair * 2 + hh, :],
                                            start=(kb == 0 and hh == 0), stop=(kb == NT - 1),
                                            skip_group_check=True,
                                        )
                            # epilogue: normalize by the ones-column sum, transpose out
                            asb = epp.tile([P, 2, DH], BF16, tag="asb")
                            for hh in range(2):
                                rec = epp.tile([P, 1], F32, tag=f"rec{hh}")
                                nc.vector.reciprocal(out=rec, in_=pv[:, hh, DH : DH + 1])
                                nc.vector.tensor_scalar_mul(
                                    out=asb[:, hh, :], in0=pv[:, hh, 0:DH],
                                    scalar1=rec,
                                )
                            nc.sync.dma_start_transpose(
                                out=attnT_all[:, pair, q0 + qs * P : q0 + (qs + 1) * P],
                                in_=asb.rearrange("p a b -> p (a b)"),
                            )
                            if j == 3 and pair == NPAIR - 1:
                                emit_outproj_tile(q0 + qs * P)
    nc.compile()
    return nc


def _rope_tables():
    inv = 1.0 / (BASE ** (np.arange(0, DH, 2, dtype=np.float32) / DH))
    t = np.arange(S, dtype=np.float32)
    freqs = t[:, None] * inv[None, :]  # [S, 32]
    bf = ml_dtypes.bfloat16
    # doubled per head: [c|c] and [s|s] over the 64-dim head
    cos_rep = np.concatenate([np.cos(freqs), np.cos(freqs)], axis=1).astype(bf)  # [S, 64]
    sin_rep = np.concatenate([np.sin(freqs), np.sin(freqs)], axis=1).astype(bf)
    return np.ascontiguousarray(cos_rep), np.ascontiguousarray(sin_rep)


def kernel(x, w_qkv, w_out, b_out, ln_gamma, ln_beta, _want_results=False, _trace=False):
    x = np.asarray(x, dtype=np.float32)
    w_qkv = np.asarray(w_qkv, dtype=np.float32)
    w_out = np.asarray(w_out, dtype=np.float32)
    b_out = np.asarray(b_out, dtype=np.float32)
    ln_gamma = np.asarray(ln_gamma, dtype=np.float32)
    ln_beta = np.asarray(ln_beta, dtype=np.float32)
    assert np.all(ln_beta == 0.0), "nonzero ln_beta not supported by this kernel"

    if "nc" not in _CACHE:
        _CACHE["nc"] = _build_nc()
    nc = _CACHE["nc"]

    wg = w_qkv * ln_gamma[:, None]  # fold gamma into the projection
    cos_rep, sin_rep = _rope_tables()
    bf = ml_dtypes.bfloat16

    in_maps = []
    for core in range(8):
        b, hg = core // HG, core % HG
        c0 = hg * IN
        in_maps.append({
            "x": np.ascontiguousarray(x[b]),
            "wq": np.ascontiguousarray(wg[:, c0 : c0 + IN]).astype(bf),
            "wk": np.ascontiguousarray(wg[:, D + c0 : D + c0 + IN]).astype(bf),
            "wv": np.ascontiguousarray(wg[:, 2 * D + c0 : 2 * D + c0 + IN]).astype(bf),
            "wo": np.ascontiguousarray(w_out[c0 : c0 + IN, :]).astype(bf),
            "cos_rep": cos_rep,
            "sin_rep": sin_rep,
        })

    res = run_bass_kernel_spmd(nc, in_maps, list(range(8)), trace=_trace)
    parts = [res.results[c]["out"] for c in range(8)]
    out = np.empty((B, S, D), dtype=np.float32)
    for b in range(B):
        out[b] = parts[2 * b] + parts[2 * b + 1] + b_out[None, :]
    if _want_results:
        return out, res
    return out


if __name__ == "__main__":
    rng = np.random.default_rng(0)
    inputs = {
        "x": rng.standard_normal((B, S, D), dtype=np.float32),
        "w_qkv": (rng.standard_normal((D, 3 * D), dtype=np.float32) * D ** -0.5),
        "w_out": (rng.standard_normal((D, D), dtype=np.float32) * D ** -0.5),
        "b_out": np.zeros(D, np.float32),
        "ln_gamma": np.ones(D, np.float32),
        "ln_beta": np.zeros(D, np.float32),
    }
    out = kernel(**inputs)
    print("ok", out.shape, out.dtype)
